# revision 11
# baseline (speedup 1.0000x reference)
"""Trainium2 Bass kernel: channel-attention MultiHeadAttention block.

Full (unsharded) inputs in, full output out. Internally: data-parallel over
batch B across 8 NeuronCores (1 batch each), with a tiny AllReduce for the
BatchNorm batch statistics.

Per-core math (batch b), all shapes [partition, free]:
  qsb/ksb/vsb   [65, 4096]   inputs + ones row (bias fold)
  wqe/wke/wve   [65, 512]    [W.T; bias]
  QT_m, KT_m    [128, 512]   projections transposed (m on partitions)
  scores        [128c, 512d] = sum_m QT[:,c-chunk].T @ KT      (4 psum tiles)
  attn          exp(scores/64) via ACT (+row sums)
  attnT         PE-transpose blocks fused with diag(1/rowsum)
  V             [128d, 4096m] natural layout
  X[cc]         [128c', 4096s'] = attn@V directly in post-permute BN layout,
                via stride-8 m-slices of V as the matmul stationary operand
  BN stats      row sums/sumsq -> AllReduce over 8 cores -> alpha/beta
  BN+leaky      in-place ACT Lrelu(scale=alpha, bias=beta)
  w1 + leaky    [512,512] conv, bias+leaky fused in ACT
  w2 + bias     [64,512] conv -> y [64, 4096]
"""

import sys

if "/opt/trn_rl_repo" not in sys.path:
    sys.path.insert(0, "/opt/trn_rl_repo")

import numpy as np

import concourse.bacc as bacc
import concourse.mybir as mybir
import concourse.tile as tile
from concourse import bass_utils

B = 8
C = 64
CN = 512
HW = 4096
NM = HW // 128   # 32 m-chunks
NCH = CN // 128  # 4 channel chunks
NS = HW // 512   # 8 free-dim slices
EPS = 1e-4
SLOPE = 0.01
INV_SCALE = 1.0 / 64.0      # 1/sqrt(HW)
INV_BHW = 1.0 / (B * HW)    # BN divisor

F32 = mybir.dt.float32
F32R = mybir.dt.float32r
AF = mybir.ActivationFunctionType
ALU = mybir.AluOpType
AX = mybir.AxisListType
RG = [[0, 1, 2, 3, 4, 5, 6, 7]]


def _r(ap):
    return ap.bitcast(F32R)


def _body(tc, nc, d, dbg=None):
    with (
        tc.tile_pool(name="consts", bufs=1) as consts,
        tc.tile_pool(name="small", bufs=1) as small,
        tc.tile_pool(name="atp", bufs=1) as atp,
        tc.tile_pool(name="vbuf", bufs=1) as vpool,
    ):
        # ---- weights / constants ----
        wqe = consts.tile([65, 512], F32R, name="wqe", tag="wqe")
        nc.sync.dma_start(wqe[:], d["wqe"][:])
        wke = consts.tile([65, 512], F32R, name="wke", tag="wke")
        nc.sync.dma_start(wke[:], d["wke"][:])
        wve = consts.tile([65, 512], F32R, name="wve", tag="wve")
        nc.sync.dma_start(wve[:], d["wve"][:])
        w1sb = []
        for cc in range(NCH):
            t = consts.tile([128, 512], F32R, name=f"w1sb{cc}", tag=f"w1sb{cc}")
            nc.sync.dma_start(t[:], d["w1t"][128 * cc:128 * (cc + 1), :])
            w1sb.append(t)
        w2sb = []
        for oc in range(NCH):
            t = consts.tile([128, 64], F32R, name=f"w2sb{oc}", tag=f"w2sb{oc}")
            nc.sync.dma_start(t[:], d["w2t"][128 * oc:128 * (oc + 1), :])
            w2sb.append(t)
        b1sb = consts.tile([128, 4], F32, name="b1sb", tag="b1sb")
        nc.sync.dma_start(b1sb[:], d["b1c"][:])
        b2sb = consts.tile([64, 1], F32, name="b2sb", tag="b2sb")
        nc.sync.dma_start(b2sb[:], d["b2c"][:])
        bngsb = consts.tile([128, 4], F32, name="bngsb", tag="bngsb")
        nc.sync.dma_start(bngsb[:], d["bng"][:])
        bnbsb = consts.tile([128, 4], F32, name="bnbsb", tag="bnbsb")
        nc.sync.dma_start(bnbsb[:], d["bnb"][:])
        ident = consts.tile([128, 128], F32R, name="ident", tag="ident")
        nc.sync.dma_start(ident[:], d["ident"][:])

        alpha = small.tile([128, 4], F32, name="alpha", tag="alpha")
        beta = small.tile([128, 4], F32, name="beta", tag="beta")
        epsb = small.tile([128, 1], F32, name="epsb", tag="epsb")
        nc.gpsimd.memset(epsb[:], EPS)

        V = [vpool.tile([128, HW], F32R, name=f"V{dc}", tag=f"V{dc}")
             for dc in range(NCH)]
        aT = [atp.tile([128, CN], F32R, name=f"aT{dc}", tag=f"aT{dc}")
              for dc in range(NCH)]

        # ================= phase 1: proj + scores + softmax + V ==========
        with (
            tc.tile_pool(name="inp", bufs=1) as inp,
            tc.tile_pool(name="qk", bufs=3) as qkp,
            tc.tile_pool(name="attn", bufs=1) as ap_,
            tc.tile_pool(name="scps", bufs=1, space="PSUM") as scps,
            tc.tile_pool(name="pjps", bufs=2, space="PSUM") as pjps,
        ):
            qsb = inp.tile([65, HW], F32R, name="qsb", tag="qsb")
            ksb = inp.tile([65, HW], F32R, name="ksb", tag="ksb")
            vsb = inp.tile([65, HW], F32R, name="vsb", tag="vsb")
            for j in range(4):
                sl = slice(1024 * j, 1024 * (j + 1))
                nc.sync.dma_start(qsb[:, sl], d["q"][:, sl])
                nc.sync.dma_start(ksb[:, sl], d["k"][:, sl])
                nc.sync.dma_start(vsb[:, sl], d["v"][:, sl])

            sc = [scps.tile([128, 512], F32, name=f"sc{cc}", tag=f"sc{cc}")
                  for cc in range(NCH)]
            for mi in range(NM):
                msl = slice(128 * mi, 128 * (mi + 1))
                qtp = pjps.tile([128, 512], F32, name="qtp", tag="qtp")
                nc.tensor.matmul(qtp[:], qsb[:, msl], wqe[:],
                                 start=True, stop=True)
                qts = qkp.tile([128, 512], F32R, name="qts", tag="qts")
                nc.scalar.copy(qts[:], qtp[:])
                ktp = pjps.tile([128, 512], F32, name="ktp", tag="ktp")
                nc.tensor.matmul(ktp[:], ksb[:, msl], wke[:],
                                 start=True, stop=True)
                kts = qkp.tile([128, 512], F32R, name="kts", tag="kts")
                nc.vector.tensor_copy(kts[:], ktp[:])
                for cc in range(NCH):
                    nc.tensor.matmul(sc[cc][:],
                                     qts[:, 128 * cc:128 * (cc + 1)],
                                     kts[:],
                                     start=(mi == 0), stop=(mi == NM - 1))

            # softmax on ACT/DVE (overlaps V projection below on PE)
            rowsum = ap_.tile([128, 4], F32, name="rowsum", tag="rowsum")
            recip = ap_.tile([128, 4], F32, name="recip", tag="recip")
            attn = [ap_.tile([128, 512], F32R, name=f"attn{cc}", tag=f"attn{cc}")
                    for cc in range(NCH)]
            for cc in range(NCH):
                nc.scalar.activation(attn[cc][:], sc[cc][:], AF.Exp,
                                     bias=0.0, scale=INV_SCALE,
                                     accum_out=rowsum[:, cc:cc + 1])

            # V projection (natural [d, m] layout)
            for dc in range(NCH):
                for ms in range(NS):
                    ssl = slice(512 * ms, 512 * (ms + 1))
                    vtp = pjps.tile([128, 512], F32, name="vtp", tag="qtp")
                    nc.tensor.matmul(vtp[:],
                                     wve[:, 128 * dc:128 * (dc + 1)],
                                     vsb[:, ssl], start=True, stop=True)
                    if (dc * NS + ms) % 2 == 0:
                        nc.scalar.copy(V[dc][:, ssl], vtp[:])
                    else:
                        nc.vector.tensor_copy(V[dc][:, ssl], vtp[:])

            if dbg is not None:
                for cc in range(NCH):
                    nc.sync.dma_start(dbg[f"attn{cc}"][:], attn[cc][:])
            # normalize rows then transpose (PE transpose ignores rhs values)
            for cc in range(NCH):
                nc.vector.reciprocal(recip[:, cc:cc + 1], rowsum[:, cc:cc + 1])
                nc.vector.tensor_scalar_mul(attn[cc][:], attn[cc][:],
                                            recip[:, cc:cc + 1])
                for dc in range(NCH):
                    tp = pjps.tile([128, 128], F32, name="tp", tag="ktp")
                    nc.tensor.transpose(_r(tp[:]),
                                        attn[cc][:, 128 * dc:128 * (dc + 1)],
                                        ident[:])
                    nc.scalar.copy(aT[dc][:, 128 * cc:128 * (cc + 1)], tp[:])

        if dbg is not None:
            for dc in range(NCH):
                nc.sync.dma_start(dbg[f"aT{dc}"][:], aT[dc][:])
                nc.sync.dma_start(dbg[f"V{dc}"][:], V[dc][:])
        # ================= phase 2: attn@V -> X (BN layout) + stats ======
        with (
            tc.tile_pool(name="xbuf", bufs=1) as xpool,
            tc.tile_pool(name="stp", bufs=2) as stp,
            tc.tile_pool(name="scr", bufs=2) as scr,
            tc.tile_pool(name="cdram", bufs=1, space="DRAM") as cdram,
        ):
            X = [xpool.tile([128, HW], F32R, name=f"X{cc}", tag=f"X{cc}")
                 for cc in range(NCH)]
            with tc.tile_pool(name="xps", bufs=6, space="PSUM") as xps:
                for cc in range(NCH):
                    Vr = [V[dc].rearrange("d (cc t lo) -> d cc lo t", cc=4, lo=8)
                          for dc in range(NCH)]
                    ps_sum = stp.tile([128, 8], F32, name="pssum", tag="pssum")
                    ps_sq = stp.tile([128, 8], F32, name="pssq", tag="pssq")
                    for lo in range(8):
                        xt = xps.tile([128, 512], F32, name="xt", tag="xt")
                        for dc in range(NCH):
                            nc.tensor.matmul(xt[:], Vr[dc][:, cc, lo, :],
                                             aT[dc][:],
                                             start=(dc == 0), stop=(dc == 3))
                        xsl = slice(512 * lo, 512 * (lo + 1))
                        nc.vector.tensor_scalar(
                            out=X[cc][:, xsl], in0=xt[:], scalar1=1.0,
                            scalar2=0.0, op0=ALU.mult, op1=ALU.add,
                            accum_out=ps_sum[:, lo:lo + 1])
                        junk = scr.tile([128, 512], F32, name="junk", tag="junk")
                        nc.scalar.activation(junk[:], xt[:], AF.Square,
                                             accum_out=ps_sq[:, lo:lo + 1])

                    red = stp.tile([128, 2], F32, name="red", tag="red")
                    nc.vector.reduce_sum(red[:, 0:1], ps_sum[:], axis=AX.X)
                    nc.vector.reduce_sum(red[:, 1:2], ps_sq[:], axis=AX.X)
                    cin = cdram.tile([128, 2], F32, name=f"cin{cc}", tag=f"cin{cc}")
                    cout = cdram.tile([128, 2], F32, name=f"cout{cc}", tag=f"cout{cc}")
                    nc.sync.dma_start(cin[:], red[:])
                    nc.gpsimd.collective_compute(
                        "AllReduce", ALU.add, replica_groups=RG,
                        ins=[cin.opt()], outs=[cout.opt()])
                    ar = stp.tile([128, 2], F32, name="ar", tag="ar")
                    nc.sync.dma_start(ar[:], cout[:])

                    # BN affine params for this chunk
                    mean = stp.tile([128, 1], F32, name="mean", tag="mean")
                    nc.vector.tensor_scalar_mul(mean[:], ar[:, 0:1], INV_BHW)
                    ex2 = stp.tile([128, 1], F32, name="ex2", tag="ex2")
                    nc.vector.tensor_scalar_mul(ex2[:], ar[:, 1:2], INV_BHW)
                    var = stp.tile([128, 1], F32, name="var", tag="var")
                    nc.vector.tensor_mul(var[:], mean[:], mean[:])
                    nc.vector.tensor_sub(var[:], ex2[:], var[:])
                    sd = stp.tile([128, 1], F32, name="sd", tag="sd")
                    nc.scalar.activation(sd[:], var[:], AF.Sqrt,
                                         bias=epsb[:, 0:1])
                    rstd = stp.tile([128, 1], F32, name="rstd", tag="rstd")
                    nc.vector.reciprocal(rstd[:], sd[:])
                    nc.vector.tensor_mul(alpha[:, cc:cc + 1],
                                         bngsb[:, cc:cc + 1], rstd[:])
                    tmp = stp.tile([128, 1], F32, name="tmpb", tag="tmpb")
                    nc.vector.tensor_mul(tmp[:], mean[:], alpha[:, cc:cc + 1])
                    nc.vector.tensor_sub(beta[:, cc:cc + 1],
                                         bnbsb[:, cc:cc + 1], tmp[:])
                    if dbg is not None:
                        nc.sync.dma_start(dbg[f"X{cc}"][:], X[cc][:])
                        nc.sync.dma_start(dbg[f"ar{cc}"][:], ar[:])
                        nc.sync.dma_start(dbg[f"ab{cc}"][:, 0:1],
                                          alpha[:, cc:cc + 1])
                        nc.sync.dma_start(dbg[f"ab{cc}"][:, 1:2],
                                          beta[:, cc:cc + 1])
                    # BN + leaky, in place
                    for lo in range(8):
                        xsl = slice(512 * lo, 512 * (lo + 1))
                        nc.scalar.activation(X[cc][:, xsl], X[cc][:, xsl],
                                             AF.Lrelu,
                                             bias=beta[:, cc:cc + 1],
                                             scale=alpha[:, cc:cc + 1],
                                             alpha=SLOPE)

            # ================= phase 3: w1 -> leaky -> w2 -> y ===========
            with (
                tc.tile_pool(name="y2", bufs=2) as y2p,
                tc.tile_pool(name="outb", bufs=1) as outp,
                tc.tile_pool(name="wps", bufs=1, space="PSUM") as wps,
                tc.tile_pool(name="w2ps", bufs=2, space="PSUM") as w2ps,
            ):
                osb = outp.tile([64, HW], F32, name="osb", tag="osb")
                for ms in range(NS):
                    ssl = slice(512 * ms, 512 * (ms + 1))
                    y2t = []
                    for oc in range(NCH):
                        wp = wps.tile([128, 512], F32, name=f"wp{oc}",
                                      tag=f"wp{oc}")
                        for cc in range(NCH):
                            nc.tensor.matmul(
                                wp[:],
                                w1sb[cc][:, 128 * oc:128 * (oc + 1)],
                                X[cc][:, ssl],
                                start=(cc == 0), stop=(cc == 3))
                        yt = y2p.tile([128, 512], F32R, name=f"y2_{oc}",
                                      tag=f"y2_{oc}")
                        nc.scalar.activation(yt[:], wp[:], AF.Lrelu,
                                             bias=b1sb[:, oc:oc + 1],
                                             scale=1.0, alpha=SLOPE)
                        y2t.append(yt)
                    fp = w2ps.tile([64, 512], F32, name="fp", tag="fp")
                    for oc in range(NCH):
                        nc.tensor.matmul(fp[:], w2sb[oc][:], y2t[oc][:],
                                         start=(oc == 0), stop=(oc == 3))
                    nc.scalar.activation(osb[:, ssl], fp[:], AF.Identity,
                                         bias=b2sb[:, 0:1])
                    nc.sync.dma_start(d["y"][:, ssl], osb[:, ssl])


_NC_CACHE = {}


def _build(debug=False):
    key = ("dbg" if debug else "nc")
    if key in _NC_CACHE:
        return _NC_CACHE[key]
    nc = bacc.Bacc(trn_type="TRN2", target_bir_lowering=False, debug=False,
                   enable_asserts=False, num_devices=8)
    d = {}
    d["q"] = nc.dram_tensor("q", (65, HW), F32R, kind="ExternalInput").ap()
    d["k"] = nc.dram_tensor("k", (65, HW), F32R, kind="ExternalInput").ap()
    d["v"] = nc.dram_tensor("v", (65, HW), F32R, kind="ExternalInput").ap()
    d["wqe"] = nc.dram_tensor("wqe", (65, 512), F32R, kind="ExternalInput").ap()
    d["wke"] = nc.dram_tensor("wke", (65, 512), F32R, kind="ExternalInput").ap()
    d["wve"] = nc.dram_tensor("wve", (65, 512), F32R, kind="ExternalInput").ap()
    d["w1t"] = nc.dram_tensor("w1t", (512, 512), F32R, kind="ExternalInput").ap()
    d["w2t"] = nc.dram_tensor("w2t", (512, 64), F32R, kind="ExternalInput").ap()
    d["b1c"] = nc.dram_tensor("b1c", (128, 4), F32, kind="ExternalInput").ap()
    d["b2c"] = nc.dram_tensor("b2c", (64, 1), F32, kind="ExternalInput").ap()
    d["bng"] = nc.dram_tensor("bng", (128, 4), F32, kind="ExternalInput").ap()
    d["bnb"] = nc.dram_tensor("bnb", (128, 4), F32, kind="ExternalInput").ap()
    d["ident"] = nc.dram_tensor("ident", (128, 128), F32R,
                                kind="ExternalInput").ap()
    d["y"] = nc.dram_tensor("y", (64, HW), F32, kind="ExternalOutput").ap()

    dbg = None
    if debug:
        dbg = {}
        for cc in range(NCH):
            dbg[f"attn{cc}"] = nc.dram_tensor(f"dbg_attn{cc}", (128, 512), F32R, kind="ExternalOutput").ap()
            dbg[f"aT{cc}"] = nc.dram_tensor(f"dbg_aT{cc}", (128, 512), F32R, kind="ExternalOutput").ap()
            dbg[f"V{cc}"] = nc.dram_tensor(f"dbg_V{cc}", (128, HW), F32R, kind="ExternalOutput").ap()
            dbg[f"X{cc}"] = nc.dram_tensor(f"dbg_X{cc}", (128, HW), F32R, kind="ExternalOutput").ap()
            dbg[f"ar{cc}"] = nc.dram_tensor(f"dbg_ar{cc}", (128, 2), F32, kind="ExternalOutput").ap()
            dbg[f"ab{cc}"] = nc.dram_tensor(f"dbg_ab{cc}", (128, 2), F32, kind="ExternalOutput").ap()
    with tile.TileContext(nc) as tc:
        _body(tc, nc, d, dbg)
    nc.compile()
    _NC_CACHE[key] = nc
    return nc


def _run(q, k, v, wq, bq, wk, bk, wv, bv, bn_g, bn_b, w1, b1, w2, b2,
         trace=False, tmpdir=None, debug=False):
    nc = _build(debug)
    f = np.float32
    shared = {
        "wqe": np.concatenate([wq.T, bq[None, :]], axis=0).astype(f),
        "wke": np.concatenate([wk.T, bk[None, :]], axis=0).astype(f),
        "wve": np.concatenate([wv.T, bv[None, :]], axis=0).astype(f),
        "w1t": np.ascontiguousarray(w1.T).astype(f),
        "w2t": np.ascontiguousarray(w2.T).astype(f),
        "b1c": np.ascontiguousarray(b1.reshape(4, 128).T).astype(f),
        "b2c": np.ascontiguousarray(b2.reshape(64, 1)).astype(f),
        "bng": np.ascontiguousarray(bn_g.reshape(4, 128).T).astype(f),
        "bnb": np.ascontiguousarray(bn_b.reshape(4, 128).T).astype(f),
        "ident": np.eye(128, dtype=f),
    }
    in_maps = []
    for b in range(B):
        m = dict(shared)
        ones = np.ones((1, HW), f)
        m["q"] = np.concatenate([q[b].reshape(64, HW), ones], axis=0).astype(f)
        m["k"] = np.concatenate([k[b].reshape(64, HW), ones], axis=0).astype(f)
        m["v"] = np.concatenate([v[b].reshape(64, HW), ones], axis=0).astype(f)
        in_maps.append(m)
    res = bass_utils.run_bass_kernel_spmd(
        nc, in_maps, core_ids=list(range(8)), trace=trace, tmpdir=tmpdir)
    out = np.stack([res.results[b]["y"].reshape(C, 64, 64) for b in range(B)])
    return out.astype(np.float32), res


def kernel(q, k, v, wq, bq, wk, bk, wv, bv, bn_g, bn_b, w1, b1, w2, b2):
    out, _ = _run(q, k, v, wq, bq, wk, bk, wv, bv, bn_g, bn_b, w1, b1, w2, b2)
    return out
